# revision 1
# baseline (speedup 1.0000x reference)
"""CRvNN forward kernel for 8x Trainium2 NeuronCores (Bass/Tile).

Strategy
--------
Pure data parallelism: batch 32 -> 4 per core; params replicated; no
collectives.  On-device state lives in TRANSPOSED layout (D=256 on partitions
as 2x128 chunks, sequence position i on the free axis, padded 514 -> 516).

Key algebraic insight: the reference's (S2 x S2) neighbor-probability
matrices are first-order linear recurrences.  With a = active*mask:

    (lnp @ x)[i] = a[i-1]*x[i-1] + (1-a[i-1])*(lnp @ x)[i-1]      (forward)
    (rnp @ x)[i] = a[i+1]*x[i+1] + (1-a[i+1])*(rnp @ x)[i+1]      (backward)
    deact[j]     = a[j]*u[j],  u[j] = tp[j+1] + (1-a[j+1])*u[j+1] (backward)

Each is ONE DVE tensor_tensor_scan per 128-partition chunk (the reference's
EPS=1e-9 inside the (1-a+EPS) products perturbs results by <1e-7 --
negligible).  The S^2 matrices are never materialized and no PE transposes
are ever needed: scans chain in transposed layout, which is exactly the
layout conv/w1/w2 want as lhsT/rhs.

Matmuls (conv 1280x256, w1 512x1024, w2 1024x1024, score matvec) run on PE in
fp32r (1 cyc/row, measured rel err ~1.5e-4; fp32 is 4 cyc/row).  fp32r
operands must be written by rounding producers (DVE/ACT ops with f32r out
dtype), which the scans/activations provide for free.  The w2 path can run
bf16 to save SBUF (env CRVNN_W2DT).  LayerNorm over D (= partitions) uses
ones-vector matmul partition reduction; row vectors (tp, active, LN stats)
are (1, 516) tiles; partition-broadcasts bounce through DRAM (DMA cannot
0-stride broadcast from SBUF).

This walrus build supports only ONE sync wait per instruction; a
post-scheduling pass splits multi-wait instructions into single-wait NOP
chains.
"""
import os
import sys
from contextlib import ExitStack

import numpy as np

sys.path.insert(0, "/opt/trn_rl_repo")

import bass_rust
import concourse.bass as bass
import concourse.mybir as mybir
from concourse.tile import TileContext

F32 = mybir.dt.float32
F32R = mybir.dt.float32r
BF16 = mybir.dt.bfloat16
AL = mybir.AluOpType
AF = mybir.ActivationFunctionType

NCORES = 8
NB = 4            # batch per core
D = 256
DC = 2            # D chunks of 128
S2 = 514
SP = 516          # padded sequence length
SPP = SP + 2      # scan-input tiles have leading+trailing zero pad columns
H = 1024
WIN = 5
EPS = 1e-9

SIM = os.environ.get("CRVNN_SIM", "0") == "1"
TRACE = os.environ.get("CRVNN_TRACE", "0") == "1"
MM_DT = os.environ.get("CRVNN_MMDT", "f32r")
W2_DT = os.environ.get("CRVNN_W2DT", "bf16")
GP_LVL = int(os.environ.get("CRVNN_GP", "1"))

NSPLITS = [(0, 512), (512, SP - 512)]

LAST_EXEC_NS = None
LAST_RES = None

_DT = {"f32": F32, "f32r": F32R, "bf16": BF16}


# --------------------------------------------------------------------------
# post-scheduling fixup: split multi-wait instructions into 1-wait NOP chains
# --------------------------------------------------------------------------
def _split_multiwaits(nc):
    counter = [0]

    def mk_nop(engine, wait):
        counter[0] += 1
        n = bass_rust.InstNoOp(name=f"WFIX-{counter[0]}", ins=[], outs=[])
        n.engine = engine
        n.sync_info = bass_rust.SyncInfo(on_wait=[wait], on_update=[])
        return n

    total = 0
    for f in nc.m.functions:
        for bb in f.blocks:
            out = []
            changed = False
            for inst in list(bb.instructions):
                si = inst.sync_info
                waits = list(si.on_wait) if (si is not None and si.on_wait) else []
                if len(waits) > 1:
                    for w in waits[:-1]:
                        out.append(mk_nop(inst.engine, w))
                    inst.sync_info = bass_rust.SyncInfo(
                        on_wait=[waits[-1]],
                        on_update=list(si.on_update) if si.on_update else [])
                    changed = True
                    total += 1
                out.append(inst)
            if changed:
                bb.instructions = out
    return total


def _bcast_ap(drow):
    """DRAM row AP (1, n) -> partition-broadcast AP (128, n)."""
    return bass.AP(tensor=drow.tensor, offset=drow.offset,
                   ap=[[0, 128]] + drow.ap[1:])


def _f32(ap):
    return ap.bitcast(F32) if ap.dtype != F32 else ap


def _build_program(n_steps, flags):
    nc = bass.Bass()
    R = _DT[MM_DT]
    W2R = _DT[W2_DT]

    seqT_in = nc.declare_dram_parameter("seqT", [NB, DC, 128, SP], F32, isOutput=False)
    mask_in = nc.declare_dram_parameter("mask", [NB, SP], F32, isOutput=False)
    selp_in = nc.declare_dram_parameter("selp", [NB, SP], F32, isOutput=False)
    act0_in = nc.declare_dram_parameter("act0", [NB, SP], F32, isOutput=False)
    nact0_in = nc.declare_dram_parameter("nact0", [NB, SP], F32, isOutput=False)
    itW_in = nc.declare_dram_parameter("itW", [D, D], F32, isOutput=False)
    convW_in = nc.declare_dram_parameter("convW", [WIN * D, D], F32, isOutput=False)
    scW_in = nc.declare_dram_parameter("scWc", [128, DC], F32, isOutput=False)
    w1W_in = nc.declare_dram_parameter("w1W", [2 * D, H], F32, isOutput=False)
    w2W_in = nc.declare_dram_parameter("w2W", [H, 4 * D], F32, isOutput=False)
    noc_in = nc.declare_dram_parameter("noc", [128, DC], F32, isOutput=False)
    ymn_in = nc.declare_dram_parameter("ymnc", [128, DC], F32, isOutput=False)
    opt_in = {}
    for nm, shape in [("itbc", [128, DC]), ("convbc", [128, DC]),
                      ("w1bc", [128, 8]), ("w2bc", [128, 8]), ("scbc", [1, 1]),
                      ("lngc", [128, DC]), ("lnbc", [128, DC])]:
        if flags.get(nm):
            opt_in[nm] = nc.declare_dram_parameter(nm, shape, F32, isOutput=False)
    out_dram = nc.declare_dram_parameter("out", [NB, DC, 128, S2], F32, isOutput=True)

    with TileContext(nc) as tc, ExitStack() as ctx:
        wpool = ctx.enter_context(tc.tile_pool(name="wpool", bufs=1))
        state = ctx.enter_context(tc.tile_pool(name="state", bufs=1))
        work = ctx.enter_context(tc.tile_pool(name="work", bufs=1))
        psum = ctx.enter_context(tc.tile_pool(name="psum", bufs=1, space="PSUM"))
        dram = ctx.enter_context(tc.tile_pool(name="dramp", bufs=1, space="DRAM"))

        # ---------------- weights -> SBUF (round to matmul dtype) -----------
        def load_w(name, dram_ap, shape, dt):
            t = wpool.tile(shape, dt, name=name)
            if dt == F32:
                nc.sync.dma_start(out=t, in_=dram_ap)
            else:
                tmp = work.tile(shape, F32, name=f"{name}_ld", tag="interT")
                nc.sync.dma_start(out=tmp, in_=dram_ap)
                nc.vector.tensor_copy(out=t, in_=tmp)
            return t

        convW_t = [load_w(f"convW{k}", convW_in.ap()[k * 128:(k + 1) * 128, :],
                          [128, D], R) for k in range(10)]
        w1W_t = [load_w(f"w1W{k}", w1W_in.ap()[k * 128:(k + 1) * 128, :],
                        [128, H], R) for k in range(4)]
        w2W_t = [load_w(f"w2W{k}", w2W_in.ap()[k * 128:(k + 1) * 128, :],
                        [128, H], W2R) for k in range(8)]
        scW_t = load_w("scWt", scW_in.ap(), [128, DC], R)
        itW_t = [wpool.tile([128, D], F32, name=f"itW{k}") for k in range(2)]
        for k in range(2):
            nc.sync.dma_start(out=itW_t[k], in_=itW_in.ap()[k * 128:(k + 1) * 128, :])

        noc = wpool.tile([128, DC], F32)
        nc.sync.dma_start(out=noc, in_=noc_in.ap())
        ymnc = wpool.tile([128, DC], F32)
        nc.sync.dma_start(out=ymnc, in_=ymn_in.ap())
        ones_f = wpool.tile([128, 1], F32)
        nc.vector.memset(ones_f, 1.0)
        ones_r = wpool.tile([128, 1], R)
        nc.vector.tensor_copy(out=ones_r, in_=ones_f)
        eps_t = wpool.tile([128, 1], F32)
        nc.vector.memset(eps_t, 1e-5)

        def load_opt(nm, shape):
            if nm not in opt_in:
                return None
            t = wpool.tile(shape, F32, name=f"{nm}_t")
            nc.sync.dma_start(out=t, in_=opt_in[nm].ap())
            return t

        itb_t = load_opt("itbc", [128, DC])
        convb_t = load_opt("convbc", [128, DC])
        w1b_t = load_opt("w1bc", [128, 8])
        w2b_t = load_opt("w2bc", [128, 8])
        scb_t = load_opt("scbc", [1, 1])
        lng_t = load_opt("lngc", [128, DC])
        lnb_t = load_opt("lnbc", [128, DC])

        # ---------------- per-batch persistent state ------------------------
        seqT = [state.tile([128, DC, SP], R, name=f"seqT{b}") for b in range(NB)]
        a_row = [state.tile([1, SP], F32, name=f"a_row{b}") for b in range(NB)]
        mask_r = [state.tile([1, SP], F32, name=f"mask_r{b}") for b in range(NB)]
        selp_r = [state.tile([1, SP], F32, name=f"selp_r{b}") for b in range(NB)]
        for b in range(NB):
            nc.sync.dma_start(out=a_row[b], in_=act0_in.ap()[b:b + 1, :])
            nc.sync.dma_start(out=mask_r[b], in_=mask_in.ap()[b:b + 1, :])
            nc.sync.dma_start(out=selp_r[b], in_=selp_in.ap()[b:b + 1, :])

        # DRAM bounce rows for partition-broadcast
        a_d = [dram.tile([1, SP], F32, name=f"a_d{b}") for b in range(NB)]
        na_d = [dram.tile([1, SP], F32, name=f"na_d{b}") for b in range(NB)]
        ltp_d = [dram.tile([1, SP], F32, name=f"ltp_d{b}") for b in range(NB)]
        rA_d = [dram.tile([1, SP], F32, name=f"rA_d{b}") for b in range(NB)]
        rB_d = [dram.tile([1, SP], F32, name=f"rB_d{b}") for b in range(NB)]
        rC_d = [dram.tile([1, SP], F32, name=f"rC_d{b}") for b in range(NB)]

        
        def work_big(name, tag, dtype=F32, bufs=None):
            return work.tile([128, DC, SP], dtype, name=name, tag=tag, bufs=bufs)

        def row(name):
            return work.tile([1, SP], F32, name=name, tag="rowW", bufs=8)

        def tiny(name):
            return work.tile([1, 1], F32, name=name, tag="tinyW", bufs=6)

        def bc_tile(name):
            return work.tile([128, SP], F32, name=name, tag="bcast", bufs=6)

        def bounce_bcast(row_sb, drow, name):
            """row (1,SP) SBUF -> DRAM -> (128,SP) broadcast tile."""
            if row_sb is not None:
                nc.sync.dma_start(out=drow, in_=row_sb)
            t = bc_tile(name)
            nc.sync.dma_start(out=t, in_=_bcast_ap(drow if not isinstance(drow, bass.AP) else drow))
            return t

        def napad_tile(name):
            """(128, SP+1) broadcast tile; data in cols 1..SP.  Col 0 is
            uninitialized -- scans read it only where multiplied by the zero
            initial state."""
            t = work.tile([128, SPP], F32, name=name, tag="nabP", bufs=2)
            nc.vector.memset(t[:, 0:SPP:SPP - 1], 0.0)
            return t

        def recip(out_r, in_r):
            nc.vector.reciprocal(out=out_r, in_=in_r)

        def tt(out, in0, in1, op, gp=False):
            eng = nc.gpsimd if (gp and GP_LVL > 0) else nc.vector
            eng.tensor_tensor(out=out, in0=in0, in1=in1, op=op)

        def mm(psum_ap, lhsT, rhs_chunks, nsl=NSPLITS):
            K = len(lhsT)
            for (o, s) in nsl:
                for k in range(K):
                    nc.tensor.matmul(psum_ap[:, o:o + s], lhsT[k],
                                     rhs_chunks[k][:, o:o + s],
                                     start=(k == 0), stop=(k == K - 1))

        def gelu_act(out, in_, bias):
            b = bias if bias is not None else 0.0
            if SIM:
                x2 = work.tile([out.shape[0], out.shape[-1]], F32, name="gx2",
                               tag="gelu_tmp", bufs=2)
                nc.scalar.activation(out=x2, in_=in_, func=AF.Square, bias=b)
                nc.vector.tensor_scalar(out=x2, in0=x2, scalar1=0.044715,
                                        scalar2=1.0, op0=AL.mult, op1=AL.add)
                u = work.tile([out.shape[0], out.shape[-1]], F32, name="gu",
                              tag="gelu_tmp2")
                if bias is not None:
                    nc.scalar.activation(out=u, in_=in_, func=AF.Identity, bias=b)
                else:
                    nc.scalar.activation(out=u, in_=in_, func=AF.Copy)
                nc.vector.tensor_tensor(out=x2, in0=x2, in1=u, op=AL.mult)
                nc.scalar.activation(out=x2, in_=x2, func=AF.Tanh,
                                     scale=0.7978845608028654)
                nc.vector.tensor_scalar(out=x2, in0=x2, scalar1=1.0,
                                        scalar2=0.5, op0=AL.add, op1=AL.mult)
                nc.vector.tensor_tensor(out=out, in0=x2, in1=u, op=AL.mult)
            else:
                nc.scalar.activation(out=out, in_=in_, func=AF.Gelu_apprx_tanh,
                                     bias=b, scale=1.0)

        def scan_fwd(out_c, nap, datap):
            """out[i] = data[i-1] + na[i-1]*out[i-1]; data pad supplies z0=0."""
            nc.vector.tensor_tensor_scan(
                out=out_c, data0=nap[:, 0:SP], data1=datap[:, 0:SP],
                initial=0.0, op0=AL.mult, op1=AL.add)

        def scan_bwd(out_c, nap, datap):
            nc.vector.tensor_tensor_scan(
                out=out_c[:, ::-1], data0=nap[:, SPP - 1:1:-1],
                data1=datap[:, SPP - 1:1:-1], initial=0.0,
                op0=AL.mult, op1=AL.add)

        # ---------------- LN stat rows (partition-axis over both chunks) ----
        def ln_rows(src_big, rdt):
            """src (128, DC, SP) of dtype rdt -> (rstd, m*rstd) rows (1, SP)."""
            ps_m = psum.tile([1, SP], F32, name="ps_m", tag="psrow", bufs=2)
            ones = ones_r if rdt != F32 else ones_f
            mm(ps_m, [ones, ones], [src_big[:, 0, :], src_big[:, 1, :]])
            sq = [work.tile([128, SP], rdt, name=f"sq{c}", tag="sq", bufs=2)
                  for c in range(DC)]
            for c in range(DC):
                nc.scalar.activation(out=sq[c], in_=src_big[:, c, :],
                                     func=AF.Square, bias=0.0)
            ps_v = psum.tile([1, SP], F32, name="ps_v", tag="psrow", bufs=2)
            mm(ps_v, [ones, ones], [sq[0], sq[1]])
            m_r = row("m_r")
            nc.scalar.activation(out=m_r, in_=ps_m, func=AF.Copy, scale=1.0 / D)
            v_r = row("v_r")
            nc.scalar.activation(out=v_r, in_=ps_v, func=AF.Copy, scale=1.0 / D)
            msq = row("msq")
            nc.vector.tensor_tensor(out=msq, in0=m_r, in1=m_r, op=AL.mult)
            nc.vector.tensor_tensor(out=v_r, in0=v_r, in1=msq, op=AL.subtract)
            nc.scalar.activation(out=v_r, in_=v_r, func=AF.Sqrt, bias=eps_t[0:1, 0:1])
            rstd = row("rstd")
            recip(rstd, v_r)
            mr = row("mr")
            nc.vector.tensor_tensor(out=mr, in0=m_r, in1=rstd, op=AL.mult)
            return rstd, mr

        def apply_ln_gated(dst_big, pre_big, rAB, rBB, rCB, gateB, b):
            """dst = rAB*pre - rBB [*lng +tpm*lnb] + rCB*seq-like source.

            rCB/gateB None => initial transform (dst = (rA*pre - rB) path only).
            """
            for c in range(DC):
                t1 = work.tile([128, SP], F32, name="t1g", tag="gelu_tmp", bufs=2)
                nc.vector.tensor_tensor(out=t1, in0=rAB, in1=pre_big[:, c, :],
                                        op=AL.mult)
                nc.vector.tensor_tensor(out=t1, in0=t1, in1=rBB, op=AL.subtract)
                if lng_t is not None:
                    nc.vector.tensor_scalar(out=t1, in0=t1,
                                            scalar1=lng_t[:, c:c + 1],
                                            scalar2=None, op0=AL.mult)
                    # + (tpm or mask) * lnb
                    nc.vector.scalar_tensor_tensor(
                        out=t1, in0=gateB, scalar=lnb_t[:, c:c + 1], in1=t1,
                        op0=AL.mult, op1=AL.add)
                if rCB is None:
                    nc.vector.tensor_copy(out=dst_big[:, c, :], in_=t1)
                else:
                    t2 = work.tile([128, SP], F32, name="t2g", tag="gelu_tmp2")
                    tt(t2, rCB, _f32(seqT[b][:, c, :]), AL.mult, gp=True)
                    nc.vector.tensor_tensor(out=dst_big[:, c, :], in0=t1, in1=t2,
                                            op=AL.add)

        # ================= initial transform ================================
        pending0 = None
        for b in range(NB):
            sA = work_big(f"sA{b}", tag="axT")
            nc.sync.dma_start(out=sA, in_=seqT_in.ap()[b].rearrange("c p i -> p c i"))
            pre = work_big(f"pre{b}", tag="compT", dtype=R, bufs=3)
            for c in range(DC):
                ps = psum.tile([128, SP], F32, name=f"ps_pre{b}{c}", tag="psmm", bufs=2)
                mm(ps, [itW_t[k][:, c * 128:(c + 1) * 128] for k in range(2)],
                   [sA[:, 0, :], sA[:, 1, :]])
                if itb_t is not None:
                    nc.scalar.activation(out=pre[:, c, :], in_=ps, func=AF.Identity,
                                         bias=itb_t[:, c:c + 1])
                else:
                    nc.scalar.activation(out=pre[:, c, :], in_=ps, func=AF.Copy)
            rstd, mr = ln_rows(pre, R)
            rA = row(f"rA0_{b}")
            nc.vector.tensor_tensor(out=rA, in0=rstd, in1=mask_r[b], op=AL.mult)
            rB = row(f"rB0_{b}")
            nc.vector.tensor_tensor(out=rB, in0=mr, in1=mask_r[b], op=AL.mult)
            rAB = bounce_bcast(rA, rA_d[b], f"rAB0_{b}")
            rBB = bounce_bcast(rB, rB_d[b], f"rBB0_{b}")
            maskB = None
            if lng_t is not None:
                maskB = bounce_bcast(None, mask_in.ap()[b:b + 1, :], f"mB0_{b}")
            if pending0 is not None:
                pb, ppre, pr = pending0
                apply_ln_gated(seqT[pb], ppre, pr[0], pr[1], None, pr[2], pb)
            pending0 = (b, pre, (rAB, rBB, maskB))
        pb, ppre, pr = pending0
        apply_ln_gated(seqT[pb], ppre, pr[0], pr[1], None, pr[2], pb)

        def emit_tail(b, tsc, comp):
                    # ---- phase D: transition prob + active update (rows) ----
                    masked = row(f"msk{b}")
                    nc.vector.tensor_tensor(out=masked, in0=tsc, in1=selp_r[b],
                                            op=AL.mult)
                    mx = tiny(f"mx{b}")
                    nc.vector.tensor_reduce(out=mx, in_=masked,
                                            axis=mybir.AxisListType.X, op=AL.max)
                    negmx = tiny(f"negmx{b}")
                    nc.vector.tensor_scalar(out=negmx, in0=mx, scalar1=0.0,
                                            scalar2=-1.0, op0=AL.max, op1=AL.mult)
                    et = row(f"et{b}")
                    nc.scalar.activation(out=et, in_=tsc, func=AF.Exp, bias=negmx)
                    nc.vector.tensor_tensor(out=et, in0=et, in1=selp_r[b], op=AL.mult)
                    en = tiny(f"en{b}")
                    nc.scalar.activation(out=en, in_=negmx, func=AF.Exp)
                    nc.vector.tensor_scalar(out=en, in0=en, scalar1=EPS, scalar2=None,
                                            op0=AL.add)
                    den = row(f"den{b}")
                    nc.vector.tensor_scalar(out=den, in0=et, scalar1=en, scalar2=None,
                                            op0=AL.add)
                    dei = row(f"dei{b}")
                    recip(dei, den)
                    den = dei
                    tp = row(f"tp{b}")
                    nc.vector.tensor_tensor(out=tp, in0=et, in1=den, op=AL.mult)
                    nc.sync.dma_start(out=ltp_d[b], in_=tp)

                    # deact scan (padded row tiles) + active update
                    nap = work.tile([1, SPP], F32, name=f"nap{b}", tag="rowP", bufs=4)
                    nc.vector.memset(nap[:, 0:SPP:SPP - 1], 0.0)
                    nc.vector.tensor_scalar(out=nap[:, 1:SP + 1], in0=a_row[b],
                                            scalar1=-1.0, scalar2=1.0,
                                            op0=AL.mult, op1=AL.add)
                    tpp = work.tile([1, SPP], F32, name=f"tpp{b}", tag="rowP", bufs=4)
                    nc.vector.memset(tpp[:, 0:SPP:SPP - 1], 0.0)
                    nc.vector.tensor_copy(out=tpp[:, 1:SP + 1], in_=tp)
                    u = row(f"u{b}")
                    nc.vector.tensor_tensor_scan(
                        out=u[:, ::-1], data0=nap[:, SPP - 1:1:-1],
                        data1=tpp[:, SPP - 1:1:-1], initial=0.0,
                        op0=AL.mult, op1=AL.add)
                    nd = row(f"nd{b}")
                    nc.vector.tensor_tensor(out=nd, in0=a_row[b], in1=u, op=AL.mult)
                    nc.vector.tensor_scalar(out=nd, in0=nd, scalar1=-1.0, scalar2=1.0,
                                            op0=AL.mult, op1=AL.add)
                    nc.vector.tensor_tensor(out=nd, in0=a_row[b], in1=nd, op=AL.mult)
                    nc.vector.tensor_scalar(out=nd, in0=nd, scalar1=0.0, scalar2=1.0,
                                            op0=AL.max, op1=AL.min)
                    nc.vector.tensor_tensor(out=a_row[b], in0=nd, in1=mask_r[b],
                                            op=AL.mult)
                    nc.sync.dma_start(out=a_d[b], in_=a_row[b])
                    nar = row(f"nar{b}")
                    nc.vector.tensor_scalar(out=nar, in0=a_row[b], scalar1=-1.0,
                                            scalar2=1.0, op0=AL.mult, op1=AL.add)
                    nc.sync.dma_start(out=na_d[b], in_=nar)

                    # ---- phase F: LN rows + gating rows ----
                    rstd, mr = ln_rows(comp, R)
                    tpm = row(f"tpm{b}")
                    nc.vector.tensor_tensor(out=tpm, in0=tp, in1=mask_r[b], op=AL.mult)
                    rA = row(f"rA{b}")
                    nc.vector.tensor_tensor(out=rA, in0=tpm, in1=rstd, op=AL.mult)
                    rB = row(f"rB{b}")
                    nc.vector.tensor_tensor(out=rB, in0=tpm, in1=mr, op=AL.mult)
                    rC = row(f"rC{b}")
                    nc.vector.tensor_tensor(out=rC, in0=mask_r[b], in1=tpm,
                                            op=AL.subtract)
                    rAB = bounce_bcast(rA, rA_d[b], f"rAB{b}")
                    rBB = bounce_bcast(rB, rB_d[b], f"rBB{b}")
                    rCB = bounce_bcast(rC, rC_d[b], f"rCB{b}")
                    tpmB = None
                    if lng_t is not None:
                        tpm_d = dram.tile([1, SP], F32, name=f"tpm_d{b}")
                        tpmB = bounce_bcast(tpm, tpm_d, f"tpmB{b}")

                    # stash for deferred phase G (applied during next batch's scans)
                    apply_ln_gated(seqT[b], comp, rAB, rBB, rCB, tpmB, b)

        # ================= main steps =======================================
        pending_tail = []
        for s in range(n_steps):
            for b in range(NB):
                # ---- phase A: broadcasts + base ----
                if s == 0:
                    aB = bc_tile(f"aB{b}")
                    nc.sync.dma_start(out=aB,
                                      in_=_bcast_ap(act0_in.ap()[b:b + 1, :]))
                    naB = napad_tile(f"naB{b}")
                    nc.sync.dma_start(out=naB[:, 1:SP + 1],
                                      in_=_bcast_ap(nact0_in.ap()[b:b + 1, :]))
                else:
                    aB = bounce_bcast(None, a_d[b], f"aB{b}")
                    naB = napad_tile(f"naB{b}")
                    nc.sync.dma_start(out=naB[:, 1:SP + 1], in_=_bcast_ap(na_d[b]))
                baseT = work_big(f"baseT{b}", tag="baseT", dtype=R)
                if s == 0:
                    for c in range(DC):
                        nc.vector.tensor_scalar(
                            out=baseT[:, c, :], in0=_f32(seqT[b][:, c, :]),
                            scalar1=noc[:, c:c + 1], scalar2=None, op0=AL.add)
                else:
                    ltpB = bounce_bcast(None, ltp_d[b], f"ltpB{b}")
                    for c in range(DC):
                        nc.vector.scalar_tensor_tensor(
                            out=baseT[:, c, :], in0=ltpB, scalar=ymnc[:, c:c + 1],
                            in1=_f32(seqT[b][:, c, :]), op0=AL.mult, op1=AL.add)
                        nc.vector.tensor_scalar(
                            out=baseT[:, c, :], in0=_f32(baseT[:, c, :]),
                            scalar1=noc[:, c:c + 1], scalar2=None, op0=AL.add)

                # ---- phase B: 5 scans ----
                def fill_ax(axt, src_big, gp=True):
                    for c in range(DC):
                        nc.vector.memset(axt[:, c, 0:SPP:SPP - 1], 0.0)
                        tt(axt[:, c, 1:SP + 1], aB, _f32(src_big[:, c, :]),
                           AL.mult, gp=gp)

                # lcT first: unblocks w1/w2 on PE while the l1/l2 chain runs
                lcT = work_big(f"lcT{b}", tag="lcT", dtype=R)
                axB = work.tile([128, DC, SPP], F32, name=f"axB{b}", tag="axT")
                fill_ax(axB, seqT[b])
                for c in range(DC):
                    scan_fwd(lcT[:, c, :], naB, axB[:, c])

                # deferred D/F/G tail of the previous batch (its tsc/comp are
                # ready by now, so these row chains run stall-free while this
                # batch's matmuls occupy PE)
                if len(pending_tail) >= 1:
                    emit_tail(*pending_tail.pop(0))

                # w1 -> gelu -> interT issued early on PE
                interT = work.tile([128, 8, SP], W2R, name=f"interT{b}",
                                   tag="interT")
                cc_rhs = [lcT[:, 0, :], lcT[:, 1, :],
                          seqT[b][:, 0, :], seqT[b][:, 1, :]]
                for hk in range(8):
                    ps = psum.tile([128, SP], F32, name=f"ps_w1{b}{hk}",
                                   tag="psmm", bufs=2)
                    mm(ps, [w1W_t[k][:, hk * 128:(hk + 1) * 128] for k in range(4)],
                       cc_rhs)
                    gelu_act(interT[:, hk, :], ps,
                             w1b_t[:, hk:hk + 1] if w1b_t is not None else None)

                fill_ax(axB, baseT)
                l1T = work_big(f"l1T{b}", tag="l1T", dtype=R)
                r1T = work_big(f"r1T{b}", tag="r1T", dtype=R)
                for c in range(DC):
                    scan_fwd(l1T[:, c, :], naB, axB[:, c])
                    scan_bwd(r1T[:, c, :], naB, axB[:, c])
                l2T = work_big(f"l2T{b}", tag="l2T", dtype=R)
                r2T = work_big(f"r2T{b}", tag="r2T", dtype=R)
                ax2 = work.tile([128, DC, SPP], F32, name=f"ax2{b}", tag="axT")
                fill_ax(ax2, l1T)
                for c in range(DC):
                    scan_fwd(l2T[:, c, :], naB, ax2[:, c])
                fill_ax(ax2, r1T)
                for c in range(DC):
                    scan_bwd(r2T[:, c, :], naB, ax2[:, c])


                # ---- phase C: conv (transposed) + score ----
                # contract in piece-readiness order so PE starts as soon as
                # baseT/l1T/r1T exist instead of waiting for the l2T chain
                piece_order = [(2, baseT), (1, l1T), (3, r1T), (0, l2T), (4, r2T)]
                gT = work_big(f"gT{b}", tag="gpar", dtype=R, bufs=2)
                for c in range(DC):
                    ps = psum.tile([128, SP], F32, name=f"ps_cv{b}{c}", tag="psmm", bufs=2)
                    lhsT, rhs = [], []
                    for w, piece in piece_order:
                        for ci in range(DC):
                            lhsT.append(convW_t[w * DC + ci][:, c * 128:(c + 1) * 128])
                            rhs.append(piece[:, ci, :])
                    mm(ps, lhsT, rhs)
                    gelu_act(gT[:, c, :], ps,
                             convb_t[:, c:c + 1] if convb_t is not None else None)
                ps_tsc = psum.tile([1, SP], F32, name=f"ps_tsc{b}", tag="psrow", bufs=2)
                mm(ps_tsc, [scW_t[:, c:c + 1] for c in range(DC)],
                   [gT[:, c, :] for c in range(DC)])
                tsc = row(f"tsc{b}")
                if scb_t is not None:
                    nc.scalar.activation(out=tsc, in_=ps_tsc, func=AF.Identity,
                                         bias=scb_t[0:1, 0:1])
                else:
                    nc.scalar.activation(out=tsc, in_=ps_tsc, func=AF.Copy)

                # ---- phase E: w2 -> gated sum ----
                comp = work_big(f"comp{b}", tag="compT", dtype=R, bufs=3)
                parT = work_big(f"parT{b}", tag="gpar", bufs=2)
                inter_lhsT = [interT[:, hk, :] for hk in range(8)]
                for g in [3, 0, 1, 2]:
                    for c in range(DC):
                        cc = g * DC + c
                        ps = psum.tile([128, SP], F32, name=f"ps_w2{b}{cc}",
                                       tag="psmm", bufs=2)
                        mm(ps, [w2W_t[hk][:, cc * 128:(cc + 1) * 128]
                                for hk in range(8)], inter_lhsT)
                        if g == 3:
                            if w2b_t is not None:
                                nc.scalar.activation(out=parT[:, c, :], in_=ps,
                                                     func=AF.Identity,
                                                     bias=w2b_t[:, cc:cc + 1])
                            else:
                                nc.scalar.activation(out=parT[:, c, :], in_=ps,
                                                     func=AF.Copy)
                        else:
                            bias = w2b_t[:, cc:cc + 1] if w2b_t is not None else 0.0
                            gate = work.tile([128, SP], F32, name=f"gate{b}",
                                             tag="gate", bufs=3)
                            nc.scalar.activation(out=gate, in_=ps, func=AF.Sigmoid,
                                                 bias=bias)
                            src = [lcT, seqT[b], parT][g]
                            if g == 0:
                                nc.vector.tensor_tensor(out=comp[:, c, :], in0=gate,
                                                        in1=_f32(src[:, c, :]),
                                                        op=AL.mult)
                            else:
                                gm = work.tile([128, SP], F32, name=f"gm{b}",
                                               tag="gelu_tmp2")
                                tt(gm, gate, _f32(src[:, c, :]), AL.mult, gp=True)
                                nc.vector.tensor_tensor(out=comp[:, c, :],
                                                        in0=_f32(comp[:, c, :]),
                                                        in1=gm, op=AL.add)

                pending_tail.append((b, tsc, comp))


        while pending_tail:
            emit_tail(*pending_tail.pop(0))

        # ---------------- output ------------------------------------------
        for b in range(NB):
            for c in range(DC):
                nc.sync.dma_start(out=out_dram.ap()[b, c],
                                  in_=_f32(seqT[b][:, c, 0:S2]))
    return nc


def _host_prep(inputs):
    f32 = np.float32
    seq = np.asarray(inputs["sequence"], f32)
    im = np.asarray(inputs["input_mask"], f32)
    START = np.asarray(inputs["START"], f32)
    END = np.asarray(inputs["END"], f32)
    yes_t = np.asarray(inputs["yes_t"], f32).reshape(-1)
    no_t = np.asarray(inputs["no_t"], f32).reshape(-1)
    N, S, Dd = seq.shape
    assert (N, S, Dd) == (32, 512, 256), (N, S, Dd)

    ones = np.ones((N, 1, 1), f32)
    zeros = np.zeros((N, 1, 1), f32)
    mask0 = np.concatenate([ones, im], 1)
    mask_no_end = np.concatenate([mask0, zeros], 1)
    mask_yes_end = np.concatenate([ones, mask0], 1)
    END_mask = mask_yes_end - mask_no_end
    seqA = np.concatenate([np.broadcast_to(START, (N, 1, Dd)), seq,
                           np.zeros((N, 1, Dd), f32)], 1)
    seqA = (END_mask * END + (1.0 - END_mask) * seqA).astype(f32)
    mask = mask_yes_end
    mask_no_start = np.concatenate([zeros, mask[:, 1:]], 1)
    last_tok = np.concatenate([END_mask[:, 1:], zeros], 1)
    selp = (mask_no_start * mask_no_end * (1.0 - last_tok)).astype(f32)

    seqT = np.zeros((N, DC, 128, SP), f32)
    for c in range(DC):
        seqT[:, c, :, :S2] = seqA[:, :, c * 128:(c + 1) * 128].transpose(0, 2, 1)
    maskP = np.zeros((N, SP), f32)
    maskP[:, :S2] = mask[:, :, 0]
    selpP = np.zeros((N, SP), f32)
    selpP[:, :S2] = selp[:, :, 0]
    actP = maskP.copy()
    nactP = (1.0 - actP).astype(f32)

    def chunk_col(v, nch):
        return np.ascontiguousarray(np.asarray(v, f32).reshape(nch, 128).T)

    host = {
        "seqT": seqT, "mask": maskP, "selp": selpP, "act0": actP, "nact0": nactP,
        "itW": np.asarray(inputs["itW"], f32),
        "convW": np.asarray(inputs["convW"], f32),
        "scWc": chunk_col(np.asarray(inputs["scW"], f32).reshape(-1), DC),
        "w1W": np.asarray(inputs["w1W"], f32),
        "w2W": np.asarray(inputs["w2W"], f32),
        "noc": chunk_col(no_t, DC),
        "ymnc": chunk_col(yes_t - no_t, DC),
    }
    flags = {
        "itbc": bool(np.any(np.asarray(inputs["itb"]))),
        "convbc": bool(np.any(np.asarray(inputs["convb"]))),
        "w1bc": bool(np.any(np.asarray(inputs["w1b"]))),
        "w2bc": bool(np.any(np.asarray(inputs["w2b"]))),
        "scbc": bool(np.any(np.asarray(inputs["scb"]))),
        "lngc": bool(np.any(np.asarray(inputs["lnb"])))
        or bool(np.any(np.asarray(inputs["lng"]) != 1.0)),
    }
    flags["lnbc"] = flags["lngc"]
    if flags["itbc"]:
        host["itbc"] = chunk_col(inputs["itb"], DC)
    if flags["convbc"]:
        host["convbc"] = chunk_col(inputs["convb"], DC)
    if flags["w1bc"]:
        host["w1bc"] = chunk_col(inputs["w1b"], 8)
    if flags["w2bc"]:
        host["w2bc"] = chunk_col(inputs["w2b"], 8)
    if flags["scbc"]:
        host["scbc"] = np.asarray(inputs["scb"], f32).reshape(1, 1)
    if flags["lngc"]:
        host["lngc"] = chunk_col(inputs["lng"], DC)
        host["lnbc"] = chunk_col(inputs["lnb"], DC)
    return host, flags


_PROG_CACHE = {}


def kernel(**inputs):
    global LAST_EXEC_NS, LAST_RES
    n_steps = int(inputs["n_steps"])
    host, flags = _host_prep(inputs)

    key = (n_steps, tuple(sorted(flags.items())), MM_DT, W2_DT, SIM, GP_LVL)
    if key not in _PROG_CACHE:
        _PROG_CACHE[key] = _build_program(n_steps, flags)
    nc = _PROG_CACHE[key]

    per_batch = {"seqT", "mask", "selp", "act0", "nact0"}
    in_maps = []
    for k in range(NCORES):
        m = {}
        for name, arr in host.items():
            if name in per_batch:
                m[name] = np.ascontiguousarray(arr[k * NB:(k + 1) * NB])
            else:
                m[name] = arr
        in_maps.append(m)

    if SIM:
        from concourse.bass_interp import CoreSim
        results = []
        for k in range(int(os.environ.get("CRVNN_SIM_CORES", "1"))):
            sim = CoreSim(nc)
            for name, v in in_maps[k].items():
                sim.tensor(name)[:] = v
            sim.simulate()
            results.append(np.array(sim.tensor("out")))
    else:
        from concourse.bass_utils import run_bass_kernel_spmd
        if not getattr(nc, "_waitfix_done", False):
            n = _split_multiwaits(nc)
            nc._waitfix_done = True
        res = run_bass_kernel_spmd(nc, in_maps, list(range(NCORES)), trace=TRACE)
        LAST_EXEC_NS = res.exec_time_ns
        LAST_RES = res
        results = [res.results[k]["out"] for k in range(NCORES)]

    full = np.zeros((32, S2, D), np.float32)
    for k, o in enumerate(results):
        for b in range(NB):
            for c in range(DC):
                full[k * NB + b, :, c * 128:(c + 1) * 128] = o[b, c].T
    return full

